# revision 4
# baseline (speedup 1.0000x reference)
"""GAT layer (nn_GATLayerAdj) Trainium2 Bass kernel, 8-core SPMD.

Reference computation (N=1024, di=do=64):
    a[i,j]  = x[j]@w_src + x[i]@w_tgt + bw        (attention logits)
    att     = softmax_j(where(adj>0, a, -1e16))
    y[i,j,:]= relu(x[j]@WfS.T + x[i]@WfT.T + bf)
    o[i,:]  = sum_j att[i,j] * y[i,j,:]

Key factorization: e[i,j] = exp(a[i,j])*M[i,j] with M = (adj>0) splits as
exp(atgt[i]+bw) * exp(asrc[j]) * M[i,j]; the row factor cancels in the
softmax, so att[i,j] = g[j]M[i,j] / sum_j g[j]M[i,j] with g = exp(asrc).
The device needs NO exp / softmax / transposes: the host uploads
e'^T[j,i] = g[j]*M[i,j] (transposed, PE-stationary-ready) and
r_t[i] = 1/sum_j e'^T[j,i] (same O(N^2) prep class as the old adjm
mask); all O(N^2 d) work runs on device.

Sharding: target-node dim i split across 8 cores (128 target rows each).

Per-core schedule (source dim j on partitions), single pass over the
8 j-chunks:
  1. u is replicated to all 128 partitions on the HOST so the device
     DMAs are plain contiguous rows (2-16KB packets) instead of 1024
     small stride-0 broadcast packets. All DMAs ride the sync HWDGE
     queue (issue cost lands on the idle sync engine), ordered by
     need-time; chunks 0/1 are processed quarter-interleaved so compute
     starts after only the first 640KB.
  2. Per chunk: z = ys_bcast + urep on DVE per 2048-col quarter
     (tensor_tensor, 2x bf16); relu per a balance table: 'A' chunks on
     ACT (two [128,4096] relus), 'D' chunks on DVE (four 2048
     tensor_scalar_max, 4x). Quarter q == PSUM row-group b.
  3. T_acc[i',(i,d)] += e'^T chunk matmuls right behind each quarter's
     relu (4 x 512-col moving slabs, 4x32 PSUM partitions via
     tile_position). One pass => PE cadence is steady (~60% duty), no
     long idle window to re-trigger the HAM half-clock gate.
  4. After the final chunk's quarter-q3 matmul for bank n2, that bank
     evacuates (scale=1/s', DVE/ACT alternating) and streams out.

Numerics: bf16 inputs to the adds/matmuls, fp32 accumulation, bf16
output (host upcasts).
"""

from contextlib import ExitStack

import numpy as np
import ml_dtypes

import concourse.bass as bass
import concourse.tile as tile
from concourse import bacc, mybir
from concourse.bass_utils import run_bass_kernel_spmd

# Lighter TileContext exit: stock emits drain + full butterfly barrier +
# sem clears + second butterfly (~11us). Engines already sync at program
# end; keep the drain (output DMA completion), a sem-only rendezvous
# before the clears, and drop the trailing barrier.
import concourse.tile as _tile_mod

if not getattr(_tile_mod, "_exit_trimmed", False):
    def _drain_and_barrier_trim(self, tick_clock, wait_clock):
        from concourse.tile import ScopedClock
        nc = self.nc
        drain_inst = nc.sync.drain()
        wait_clock.add_sem_waits(
            drain_inst.ins, ScopedClock({None: tick_clock.global_clock})
        )
        exit_sem = nc.alloc_semaphore("exit_rdv")
        for eng in (nc.sync, nc.tensor, nc.vector, nc.scalar):
            eng.nop(nofuse=True).then_inc(exit_sem, 1)
        nc.gpsimd.wait_ge(exit_sem, 4)
        assert self.sems is not None
        popped = nc._tile_sem_poison_stack.pop()
        assert popped is self._sem_poison
        nc.clear_and_free_semaphores(list(self.sems.allocated().values()))
        nc.gpsimd.sem_clear(range(exit_sem.num, exit_sem.num + 1))

    _tile_mod.TileContext._drain_and_barrier = _drain_and_barrier_trim
    _tile_mod._exit_trimmed = True

N = 1024
DI = 64
DO = 64
N_CORES = 8
ROWS = N // N_CORES          # 128 target rows per core
NCHUNK = N // 128            # 8 j-chunks
F_FULL = ROWS * DO           # 8192 free size of (i, d)
HALF = F_FULL // 2           # 4096
QUART = F_FULL // 4          # 2048

f32 = mybir.dt.float32
bf16 = mybir.dt.bfloat16
AF = mybir.ActivationFunctionType
ALU = mybir.AluOpType

# head blob: [ysjp 512 | urep cols 0:2048]
BLOB_W = NCHUNK * DO + QUART

# Per-chunk relu engine: 'A' = two ACT [128,4096] relus, 'D' = four DVE
# tensor_scalar_max [128,2048] (4x), 'M' = mixed (3 quarters ACT as
# half+single, quarter q3 on DVE). 9 DVE quarters total; last chunk is
# D so the tail chain (relu -> final matmuls -> evac -> out) is short.
# Balance: DVE = 32x1.13 + 9x0.68 + evac ~= 43.1; ACT = 11.5 halves
# ~= 43.5us.
CHUNK_ENG = ["A", "A", "D", "A", "A", "M", "A", "D"]

_CACHE = {}


def _build_program():
    nc = bacc.Bacc("TRN2", target_bir_lowering=False, debug=False,
                   num_devices=N_CORES)

    # ---- DRAM I/O ----
    blob_d = nc.dram_tensor("blob", [128, BLOB_W], bf16,
                            kind="ExternalInput").ap()
    urest_d = nc.dram_tensor("urest", [128, F_FULL - QUART], bf16,
                             kind="ExternalInput").ap()
    etp_d = nc.dram_tensor("etp", [128, N], bf16,
                           kind="ExternalInput").ap()
    rinv_d = nc.dram_tensor("rinv", [128, 1], f32, kind="ExternalInput").ap()
    o_d = nc.dram_tensor("o", [128, 2048], bf16, kind="ExternalOutput").ap()

    with tile.TileContext(nc) as tc, ExitStack() as ctx:
        cons = ctx.enter_context(tc.tile_pool(name="cons", bufs=1))
        zp = ctx.enter_context(tc.tile_pool(name="zp", bufs=4))
        rp = ctx.enter_context(tc.tile_pool(name="rp", bufs=6))
        accp = ctx.enter_context(tc.tile_pool(name="accp", bufs=1, space="PSUM"))

        blob = cons.tile([128, BLOB_W], bf16)
        urest = cons.tile([128, F_FULL - QUART], bf16)
        etp = cons.tile([128, N], bf16)
        r_t = cons.tile([ROWS, 1], f32)

        # ---- DMAs, all on the sync HWDGE queue, ordered by need-time:
        # head blob (ysjp + urep q0), urep q1, etp, rinv, urep q2, q3.
        nc.sync.dma_start(blob[:], blob_d[:, :])
        nc.sync.dma_start(urest[:, 0:QUART], urest_d[:, 0:QUART])
        nc.sync.dma_start(etp[:], etp_d[:, :])
        nc.sync.dma_start(r_t[:], rinv_d[:, :])
        nc.sync.dma_start(urest[:, QUART:2 * QUART],
                          urest_d[:, QUART:2 * QUART])
        nc.sync.dma_start(urest[:, 2 * QUART:], urest_d[:, 2 * QUART:])

        ys_jp = blob[:, 0:NCHUNK * DO]
        et_all = etp[:, 0:N]

        def urep(q):
            # urep quarter q as an AP: q0 lives in the blob, q1-3 in urest
            if q == 0:
                return blob[:, NCHUNK * DO:NCHUNK * DO + QUART]
            return urest[:, QUART * (q - 1):QUART * q]

        t_accs = [accp.tile([128, 512], f32, tag=f"acc{n2}", name=f"t_acc{n2}")
                  for n2 in range(4)]
        t_sb = cons.tile([128, 2048], bf16)

        def emit_adds(c, z, q):
            # z half-tile h = q//2, quarter written at [2048*(q%2) ...]
            ys_c = ys_jp[:, DO * c:DO * (c + 1)]
            zl = slice(QUART * (q % 2), QUART * (q % 2) + QUART)
            ys_b = ys_c.rearrange("p d -> p () d").broadcast_to(
                (128, QUART // DO, DO))
            zv = z[:, zl].rearrange("p (i d) -> p i d", i=QUART // DO)
            uv = urep(q).rearrange("p (i d) -> p i d", i=QUART // DO)
            nc.vector.tensor_tensor(zv, ys_b, uv, ALU.add)

        def emit_relu(c, z, h, eng):
            # returns list of (r_tile, base_col) covering quarters 2h,2h+1
            if eng == "A" or (eng == "M" and h == 0):
                r = rp.tile([128, HALF], bf16, name="rA")
                nc.scalar.activation(r[:], z[:], AF.Relu)
                return [(r, 0), (r, QUART)]
            if eng == "M":
                # q2 on ACT, q3 on DVE
                r0 = rp.tile([128, QUART], bf16, name="rq")
                r1 = rp.tile([128, QUART], bf16, name="rq")
                nc.scalar.activation(r0[:], z[:, 0:QUART], AF.Relu)
                nc.vector.tensor_scalar_max(r1[:], z[:, QUART:], 0.0)
                return [(r0, 0), (r1, 0)]
            r0 = rp.tile([128, QUART], bf16, name="rq")
            r1 = rp.tile([128, QUART], bf16, name="rq")
            nc.vector.tensor_scalar_max(r0[:], z[:, 0:QUART], 0.0)
            nc.vector.tensor_scalar_max(r1[:], z[:, QUART:], 0.0)
            return [(r0, 0), (r1, 0)]

        def emit_reduce(c, q, r, base, first, last):
            b = q
            for n2 in range(4):
                nc.tensor.matmul(
                    t_accs[n2][32 * b:32 * (b + 1), :],
                    et_all[:, 128 * c + 32 * b:128 * c + 32 * (b + 1)],
                    r[:, base + 512 * n2:base + 512 * (n2 + 1)],
                    start=first,
                    stop=last,
                    skip_group_check=True,
                    tile_position=(0, 32 * b),
                )
                if last:
                    # bank n2 fully accumulated: scaled evacuation
                    # (DVE/ACT alternating) + output DMA on sync queue
                    osl = slice(512 * n2, 512 * (n2 + 1))
                    if n2 % 2 == 0:
                        nc.vector.tensor_scalar_mul(t_sb[:, osl],
                                                    t_accs[n2][:, :], r_t[:])
                    else:
                        nc.scalar.activation(t_sb[:, osl], t_accs[n2][:, :],
                                             AF.Copy, bias=0.0, scale=r_t[:])
                    nc.sync.dma_start(out=o_d[:, osl], in_=t_sb[:, osl])

        def emit_chunk_halves(c, hs):
            eng = CHUNK_ENG[c]
            for h in hs:
                z = zp.tile([128, HALF], bf16, name="z")
                emit_adds(c, z, 2 * h)
                emit_adds(c, z, 2 * h + 1)
                parts = emit_relu(c, z, h, eng)
                for qq in range(2):
                    q = 2 * h + qq
                    r, base = parts[qq]
                    emit_reduce(c, q, r, base, first=(c == 0),
                                last=(c == NCHUNK - 1 and True))

        # chunks 0/1 interleaved at half granularity so the first adds
        # only wait for the head blob; chunk 1's low half reuses urep
        # q0/q1 while q2/q3 are still in flight.
        emit_chunk_halves(0, [0])
        emit_chunk_halves(1, [0])
        emit_chunk_halves(0, [1])
        emit_chunk_halves(1, [1])
        for c in range(2, NCHUNK):
            emit_chunk_halves(c, [0, 1])

    nc.compile()
    return nc


def _prep_inputs(x, adj, Wf, bf_, Ww, bw):
    b = ml_dtypes.bfloat16
    x64 = x.astype(np.float64)
    ys = (x64 @ Wf[:, :DI].astype(np.float64).T).astype(np.float32)   # [N, 64]
    u = (x64 @ Wf[:, DI:].astype(np.float64).T + bf_).astype(np.float32)
    asrc = (x64 @ Ww[0, :DI].astype(np.float64)).astype(np.float32)   # [N]
    g = np.exp(asrc.astype(np.float64)).astype(np.float32)            # [N]

    # ysjp[jl, 64c+d] = ys[128c+jl, d]
    ysjp = ys.reshape(NCHUNK, 128, DO).transpose(1, 0, 2).reshape(128, -1)
    # e'^T[j, i] = g[j] * (adj[i, j] > 0), chunk-packed:
    # etp[jl, 128c+il] = e'^T[128c+jl, il]
    mask_t = (adj > 0).T.astype(np.float32)          # [j, i]
    et_full = mask_t * g[:, None]                    # [j, i]
    sfull = et_full.sum(axis=0)                      # [i] row sums (denom)

    in_maps = []
    for c in range(N_CORES):
        blk = slice(ROWS * c, ROWS * (c + 1))
        et = et_full[:, blk]                          # [1024, 128]
        etp = et.reshape(NCHUNK, 128, ROWS).transpose(1, 0, 2).reshape(128, -1)
        uflat = u[blk].reshape(F_FULL).astype(b)      # [8192]
        ubc = np.broadcast_to(uflat, (128, F_FULL))   # host-side replicate
        blob = np.empty((128, BLOB_W), b)
        blob[:, 0:NCHUNK * DO] = ysjp.astype(b)
        blob[:, NCHUNK * DO:] = ubc[:, 0:QUART]
        m = dict(
            blob=blob,
            urest=np.ascontiguousarray(ubc[:, QUART:]),
            etp=np.ascontiguousarray(etp).astype(b),
            rinv=np.ascontiguousarray(
                (1.0 / sfull[blk]).reshape(128, 1)).astype(np.float32),
        )
        in_maps.append(m)
    return in_maps


def get_program():
    if "nc" not in _CACHE:
        _CACHE["nc"] = _build_program()
    return _CACHE["nc"]


def unpack_output(res_list):
    p_idx = np.arange(128)
    col0 = (p_idx % 32) * DO
    cols = col0[:, None] + np.arange(DO)[None, :]
    out = np.empty((N, DO), np.float32)
    for c in range(N_CORES):
        t = res_list[c]["o"].astype(np.float32)      # [128, 2048]
        out[ROWS * c:ROWS * (c + 1)] = t[p_idx[:, None], cols]
    return out


def kernel(x, adj, Wf, bf, Ww, bw):
    x = np.asarray(x, dtype=np.float32)
    adj = np.asarray(adj, dtype=np.int32)
    Wf = np.asarray(Wf, dtype=np.float32)
    bf_ = np.asarray(bf, dtype=np.float32)
    Ww = np.asarray(Ww, dtype=np.float32)
    bw = np.asarray(bw, dtype=np.float32)
    assert x.shape == (N, DI) and adj.shape == (N, N)

    nc = get_program()
    in_maps = _prep_inputs(x, adj, Wf, bf_, Ww, bw)
    res = run_bass_kernel_spmd(nc, in_maps, core_ids=list(range(N_CORES)))
    return unpack_output(res.results)


# revision 10
# speedup vs baseline: 1.1003x; 1.1003x over previous
"""GAT layer (nn_GATLayerAdj) Trainium2 Bass kernel, 8-core SPMD.

Reference computation (N=1024, di=do=64):
    a[i,j]  = x[j]@w_src + x[i]@w_tgt + bw        (attention logits)
    att     = softmax_j(where(adj>0, a, -1e16))
    y[i,j,:]= relu(x[j]@WfS.T + x[i]@WfT.T + bf)
    o[i,:]  = sum_j att[i,j] * y[i,j,:]

Key factorization: e[i,j] = exp(a[i,j])*M[i,j] with M = (adj>0) splits as
exp(atgt[i]+bw) * exp(asrc[j]) * M[i,j]; the row factor cancels in the
softmax, so att[i,j] = g[j]M[i,j] / sum_j g[j]M[i,j] with g = exp(asrc).
The device needs NO exp / softmax / transposes: the host uploads
e'^T[j,i] = g[j]*M[i,j] (transposed, PE-stationary-ready) and
r_t[i] = 1/sum_j e'^T[j,i] (same O(N^2) prep class as the old adjm
mask); all O(N^2 d) work runs on device.

Sharding: target-node dim i split across 8 cores (128 target rows each).

Per-core schedule (source dim j on partitions), QUARTER-PASS order:
pass q processes free columns [2048q, 2048q+2048) of all 8 chunks, so
u-broadcast slices are consumed strictly in arrival order and no
mid-kernel DMA wait occurs.
  1. u is replicated to all 128 partitions on the HOST so device DMAs
     are plain contiguous rows. DMAs ride three queues by need-time:
     sync HWDGE (head blob + u quarters 0-1 + outputs), act HWDGE
     (etp + rinv + u quarter 2), gpsimd SWDGE (u quarter 3).
  2. Per (chunk, quarter): z = ys_bcast + urep on DVE (tensor_tensor,
     2x bf16, [128,2048]); relu per a balance table on DVE
     (tensor_scalar_max, 4x) or ACT; then 4 reduce matmuls
     (b-group = q, 4x32 PSUM partitions via tile_position).
  3. A filler matmul (result discarded into a spare PSUM bank, operand
     = the freshly added z) after each quarter keeps the PE's HAM
     clock-gate warm: without it the PE idles >3us between matmul
     groups and drops to half clock for the rest of the kernel.
  4. After the final pass's chunk-7 matmul for bank n2, that bank
     evacuates (scale=1/s', DVE/ACT alternating) and streams out.

Numerics: bf16 inputs to the adds/matmuls, fp32 accumulation, bf16
output (host upcasts).
"""

from contextlib import ExitStack

import numpy as np
import ml_dtypes

import concourse.bass as bass
import concourse.tile as tile
from concourse import bacc, mybir
from concourse.bass_utils import run_bass_kernel_spmd

# Lighter TileContext exit: stock emits drain + full butterfly barrier +
# sem clears + second butterfly (~11us). Engines already sync at program
# end; keep the drain (output DMA completion), a sem-only rendezvous
# before the clears, and drop the trailing barrier.
import concourse.tile as _tile_mod

if not getattr(_tile_mod, "_exit_trimmed", False):
    def _drain_and_barrier_trim(self, tick_clock, wait_clock):
        from concourse.tile import ScopedClock
        nc = self.nc
        drain_inst = nc.sync.drain()
        wait_clock.add_sem_waits(
            drain_inst.ins, ScopedClock({None: tick_clock.global_clock})
        )
        exit_sem = nc.alloc_semaphore("exit_rdv")
        for eng in (nc.sync, nc.tensor, nc.vector, nc.scalar):
            eng.nop(nofuse=True).then_inc(exit_sem, 1)
        nc.gpsimd.wait_ge(exit_sem, 4)
        assert self.sems is not None
        popped = nc._tile_sem_poison_stack.pop()
        assert popped is self._sem_poison
        nc.clear_and_free_semaphores(list(self.sems.allocated().values()))
        nc.gpsimd.sem_clear(range(exit_sem.num, exit_sem.num + 1))

    _tile_mod.TileContext._drain_and_barrier = _drain_and_barrier_trim
    _tile_mod._exit_trimmed = True

N = 1024
DI = 64
DO = 64
N_CORES = 8
ROWS = N // N_CORES          # 128 target rows per core
NCHUNK = N // 128            # 8 j-chunks
F_FULL = ROWS * DO           # 8192 free size of (i, d)
QUART = F_FULL // 4          # 2048

f32 = mybir.dt.float32
bf16 = mybir.dt.bfloat16
AF = mybir.ActivationFunctionType
ALU = mybir.AluOpType

YW = NCHUNK * DO             # 512 ysjp cols
# head blob: [ysjp 512 | urep cols 0:512]
BLOB_W = YW + 512

# Relu engine per (pass q, chunk c): 'D' = DVE tensor_scalar_max
# [128,2048] (4x), 'A' = ACT [128,2048] relu. 11 DVE quarters, 21 ACT.
# Last quarter of the kernel is D so the tail chain is short. Balance:
# DVE = 32x1.14 + 11x0.68 + evac ~= 44.5; ACT = 21x2.0 + evac ~= 43.3.
RELU_ENG = [
    "ADAADAAD",
    "AADAADAA",
    "DAADAADA",
    "ADAADAAD",
]

_CACHE = {}


def _build_program():
    nc = bacc.Bacc("TRN2", target_bir_lowering=False, debug=False,
                   num_devices=N_CORES)

    # ---- DRAM I/O ----
    blob_d = nc.dram_tensor("blob", [128, BLOB_W], bf16,
                            kind="ExternalInput").ap()
    u0_d = nc.dram_tensor("u0", [128, 512], bf16, kind="ExternalInput").ap()
    u1_d = nc.dram_tensor("u1", [128, 1536], bf16, kind="ExternalInput").ap()
    u2_d = nc.dram_tensor("u2", [128, 2048], bf16, kind="ExternalInput").ap()
    u3_d = nc.dram_tensor("u3", [128, 2048], bf16, kind="ExternalInput").ap()
    u4_d = nc.dram_tensor("u4", [128, 2048], bf16, kind="ExternalInput").ap()
    etp_d = nc.dram_tensor("etp", [128, N], bf16,
                           kind="ExternalInput").ap()
    rinv_d = nc.dram_tensor("rinv", [128, 1], f32, kind="ExternalInput").ap()
    o_d = nc.dram_tensor("o", [128, 2048], bf16, kind="ExternalOutput").ap()

    with tile.TileContext(nc) as tc, ExitStack() as ctx:
        cons = ctx.enter_context(tc.tile_pool(name="cons", bufs=1))
        zp = ctx.enter_context(tc.tile_pool(name="zp", bufs=4))
        rp = ctx.enter_context(tc.tile_pool(name="rp", bufs=6))
        accp = ctx.enter_context(tc.tile_pool(name="accp", bufs=1, space="PSUM"))

        blob = cons.tile([128, BLOB_W], bf16)
        urep = cons.tile([128, F_FULL], bf16)
        etp = cons.tile([128, N], bf16)
        r_t = cons.tile([ROWS, 1], f32)

        # ---- DMAs on three queues, ordered by need-time. u cols 0:512
        # are uploaded twice (blob for chunk 0's first sub-adds, urep
        # for the rest) so every quarter AP stays within one tile.
        nc.sync.dma_start(blob[:], blob_d[:, :])
        nc.sync.dma_start(urep[:, 0:512], u0_d[:, :])
        nc.sync.dma_start(urep[:, 512:2048], u1_d[:, :])
        nc.sync.dma_start(urep[:, 2048:4096], u2_d[:, :])
        nc.scalar.dma_start(etp[:], etp_d[:, :])
        nc.scalar.dma_start(r_t[:], rinv_d[:, :])
        nc.scalar.dma_start(urep[:, 4096:6144], u3_d[:, :])
        nc.gpsimd.dma_start(urep[:, 6144:8192], u4_d[:, :])

        ys_jp = blob[:, 0:YW]
        et_all = etp[:, 0:N]

        def usl(c0, c1, from_blob=False):
            # u columns [c0, c1): chunk 0's first sub-adds read the
            # early blob copy, everything else the full urep tile
            if from_blob and c1 <= 512:
                return blob[:, YW + c0:YW + c1]
            return urep[:, c0:c1]

        t_accs = [accp.tile([128, 512], f32, tag=f"acc{n2}", name=f"t_acc{n2}")
                  for n2 in range(4)]
        fill_b = accp.tile([128, 512], f32, tag="fill", name="fill_b")
        t_sb = cons.tile([128, 2048], bf16)

        def emit_add(c, z, q, parts):
            # z[:, zl] = ys_c (bcast over i) + u[qcols], in sub-steps
            ys_c = ys_jp[:, DO * c:DO * (c + 1)]
            pos = 0
            for step in parts:
                sl = (QUART * q + pos, QUART * q + pos + step)
                ys_b = ys_c.rearrange("p d -> p () d").broadcast_to(
                    (128, step // DO, DO))
                zv = z[:, pos:pos + step].rearrange(
                    "p (i d) -> p i d", i=step // DO)
                uv = usl(*sl, from_blob=(c == 0 and q == 0)).rearrange(
                    "p (i d) -> p i d", i=step // DO)
                nc.vector.tensor_tensor(zv, ys_b, uv, ALU.add)
                pos += step

        def emit_quarter(q, c, first, last):
            z = zp.tile([128, QUART], bf16, name="z")
            subs = (512, 512, 1024) if (q, c) == (0, 0) else (QUART,)
            emit_add(c, z, q, subs)
            # PE keep-warm filler: fires as soon as z (pre-relu) exists,
            # bridging the idle window while the relu runs.
            nc.tensor.matmul(fill_b[0:32, :], et_all[:, 0:32], z[:, 0:512],
                             start=True, stop=True, skip_group_check=True)
            r = rp.tile([128, QUART], bf16, name="r")
            if RELU_ENG[q][c] == "D":
                nc.vector.tensor_scalar_max(r[:], z[:], 0.0)
            else:
                nc.scalar.activation(r[:], z[:], AF.Relu)
            for n2 in range(4):
                nc.tensor.matmul(
                    t_accs[n2][32 * q:32 * (q + 1), :],
                    et_all[:, 128 * c + 32 * q:128 * c + 32 * q + 32],
                    r[:, 512 * n2:512 * (n2 + 1)],
                    start=first,
                    stop=last,
                    skip_group_check=True,
                    tile_position=(0, 32 * q),
                )
                if last and q == 3:
                    # bank n2 fully accumulated: scaled evacuation
                    # (DVE/ACT alternating) + output DMA on sync queue
                    osl = slice(512 * n2, 512 * (n2 + 1))
                    if n2 % 2 == 0:
                        nc.vector.tensor_scalar_mul(t_sb[:, osl],
                                                    t_accs[n2][:, :], r_t[:])
                    else:
                        nc.scalar.activation(t_sb[:, osl], t_accs[n2][:, :],
                                             AF.Copy, bias=0.0, scale=r_t[:])
                    nc.sync.dma_start(out=o_d[:, osl], in_=t_sb[:, osl])

        for q in range(4):
            for c in range(NCHUNK):
                emit_quarter(q, c, first=(c == 0), last=(c == NCHUNK - 1))

    nc.compile()
    return nc


def _prep_inputs(x, adj, Wf, bf_, Ww, bw):
    b = ml_dtypes.bfloat16
    x64 = x.astype(np.float64)
    ys = (x64 @ Wf[:, :DI].astype(np.float64).T).astype(np.float32)   # [N, 64]
    u = (x64 @ Wf[:, DI:].astype(np.float64).T + bf_).astype(np.float32)
    asrc = (x64 @ Ww[0, :DI].astype(np.float64)).astype(np.float32)   # [N]
    g = np.exp(asrc.astype(np.float64)).astype(np.float32)            # [N]

    # ysjp[jl, 64c+d] = ys[128c+jl, d]
    ysjp = ys.reshape(NCHUNK, 128, DO).transpose(1, 0, 2).reshape(128, -1)
    # e'^T[j, i] = g[j] * (adj[i, j] > 0), chunk-packed:
    # etp[jl, 128c+il] = e'^T[128c+jl, il]
    mask_t = (adj > 0).T.astype(np.float32)          # [j, i]
    et_full = mask_t * g[:, None]                    # [j, i]
    sfull = et_full.sum(axis=0)                      # [i] row sums (denom)

    in_maps = []
    for c in range(N_CORES):
        blk = slice(ROWS * c, ROWS * (c + 1))
        et = et_full[:, blk]                          # [1024, 128]
        etp = et.reshape(NCHUNK, 128, ROWS).transpose(1, 0, 2).reshape(128, -1)
        uflat = u[blk].reshape(F_FULL).astype(b)      # [8192]
        ubc = np.ascontiguousarray(
            np.broadcast_to(uflat, (128, F_FULL)))    # host-side replicate
        blob = np.empty((128, BLOB_W), b)
        blob[:, 0:YW] = ysjp.astype(b)
        blob[:, YW:] = ubc[:, 0:512]
        m = dict(
            blob=blob,
            u0=np.ascontiguousarray(ubc[:, 0:512]),
            u1=np.ascontiguousarray(ubc[:, 512:2048]),
            u2=np.ascontiguousarray(ubc[:, 2048:4096]),
            u3=np.ascontiguousarray(ubc[:, 4096:6144]),
            u4=np.ascontiguousarray(ubc[:, 6144:8192]),
            etp=np.ascontiguousarray(etp).astype(b),
            rinv=np.ascontiguousarray(
                (1.0 / sfull[blk]).reshape(128, 1)).astype(np.float32),
        )
        in_maps.append(m)
    return in_maps


def get_program():
    if "nc" not in _CACHE:
        _CACHE["nc"] = _build_program()
    return _CACHE["nc"]


def unpack_output(res_list):
    p_idx = np.arange(128)
    col0 = (p_idx % 32) * DO
    cols = col0[:, None] + np.arange(DO)[None, :]
    out = np.empty((N, DO), np.float32)
    for c in range(N_CORES):
        t = res_list[c]["o"].astype(np.float32)      # [128, 2048]
        out[ROWS * c:ROWS * (c + 1)] = t[p_idx[:, None], cols]
    return out


def kernel(x, adj, Wf, bf, Ww, bw):
    x = np.asarray(x, dtype=np.float32)
    adj = np.asarray(adj, dtype=np.int32)
    Wf = np.asarray(Wf, dtype=np.float32)
    bf_ = np.asarray(bf, dtype=np.float32)
    Ww = np.asarray(Ww, dtype=np.float32)
    bw = np.asarray(bw, dtype=np.float32)
    assert x.shape == (N, DI) and adj.shape == (N, N)

    nc = get_program()
    in_maps = _prep_inputs(x, adj, Wf, bf_, Ww, bw)
    res = run_bass_kernel_spmd(nc, in_maps, core_ids=list(range(N_CORES)))
    return unpack_output(res.results)


# revision 13
# speedup vs baseline: 1.1533x; 1.0481x over previous
"""GAT layer (nn_GATLayerAdj) Trainium2 Bass kernel, 8-core SPMD.

Reference computation (N=1024, di=do=64):
    a[i,j]  = x[j]@w_src + x[i]@w_tgt + bw        (attention logits)
    att     = softmax_j(where(adj>0, a, -1e16))
    y[i,j,:]= relu(x[j]@WfS.T + x[i]@WfT.T + bf)
    o[i,:]  = sum_j att[i,j] * y[i,j,:]

Key factorization: e[i,j] = exp(a[i,j])*M[i,j] with M = (adj>0) splits as
exp(atgt[i]+bw) * exp(asrc[j]) * M[i,j]; the row factor cancels in the
softmax, so att[i,j] = g[j]M[i,j] / sum_j g[j]M[i,j] with g = exp(asrc).
The device needs NO exp / softmax / transposes: the host uploads
e'^T[j,i] = g[j]*M[i,j] (transposed, PE-stationary-ready) and
r_t[i] = 1/sum_j e'^T[j,i] (same O(N^2) prep class as the old adjm
mask); all O(N^2 d) work runs on device.

Sharding: target-node dim i split across 8 cores (128 target rows each).

Per-core schedule (source dim j on partitions), QUARTER-PASS order:
pass q processes free columns [2048q, 2048q+2048) of all 8 chunks, so
u-broadcast slices are consumed strictly in arrival order and no
mid-kernel DMA wait occurs.
  1. u is replicated to all 128 partitions on the HOST so device DMAs
     are plain contiguous rows. DMAs ride three queues by need-time:
     sync HWDGE (head blob + u quarters 0-1 + outputs), act HWDGE
     (etp + rinv + u quarter 2), gpsimd SWDGE (u quarter 3).
  2. Per (chunk, quarter): z = ys_bcast + urep on DVE (tensor_tensor,
     2x bf16, [128,2048]); relu per a balance table on DVE
     (tensor_scalar_max, 4x) or ACT; then 4 reduce matmuls
     (b-group = q, 4x32 PSUM partitions via tile_position).
  3. A filler matmul (result discarded into a spare PSUM bank, operand
     = the freshly added z) after each quarter keeps the PE's HAM
     clock-gate warm: without it the PE idles >3us between matmul
     groups and drops to half clock for the rest of the kernel.
  4. After the final pass's chunk-7 matmul for bank n2, that bank
     evacuates (scale=1/s', DVE/ACT alternating) and streams out.

Numerics: bf16 inputs to the adds/matmuls, fp32 accumulation, bf16
output (host upcasts).
"""

from contextlib import ExitStack

import numpy as np
import ml_dtypes

import concourse.bass as bass
import concourse.tile as tile
from concourse import bacc, mybir
from concourse.bass_utils import run_bass_kernel_spmd

# Lighter TileContext exit: stock emits drain + full butterfly barrier +
# sem clears + second butterfly (~11us). Engines already sync at program
# end; keep the drain (output DMA completion), a sem-only rendezvous
# before the clears, and drop the trailing barrier.
import concourse.tile as _tile_mod

if not getattr(_tile_mod, "_exit_trimmed", False):
    def _drain_and_barrier_trim(self, tick_clock, wait_clock):
        from concourse.tile import ScopedClock
        nc = self.nc
        drain_inst = nc.sync.drain()
        wait_clock.add_sem_waits(
            drain_inst.ins, ScopedClock({None: tick_clock.global_clock})
        )
        exit_sem = nc.alloc_semaphore("exit_rdv")
        for eng in (nc.sync, nc.tensor, nc.vector, nc.scalar):
            eng.nop(nofuse=True).then_inc(exit_sem, 1)
        nc.gpsimd.wait_ge(exit_sem, 4)
        assert self.sems is not None
        popped = nc._tile_sem_poison_stack.pop()
        assert popped is self._sem_poison
        nc.clear_and_free_semaphores(list(self.sems.allocated().values()))
        nc.gpsimd.sem_clear(range(exit_sem.num, exit_sem.num + 1))

    _tile_mod.TileContext._drain_and_barrier = _drain_and_barrier_trim
    _tile_mod._exit_trimmed = True

N = 1024
DI = 64
DO = 64
N_CORES = 8
ROWS = N // N_CORES          # 128 target rows per core
NCHUNK = N // 128            # 8 j-chunks
F_FULL = ROWS * DO           # 8192 free size of (i, d)
QUART = F_FULL // 4          # 2048

f32 = mybir.dt.float32
bf16 = mybir.dt.bfloat16
AF = mybir.ActivationFunctionType
ALU = mybir.AluOpType

YW = NCHUNK * DO             # 512 ysjp cols
# head blob: [ysjp 512 | urep cols 0:512]
BLOB_W = YW + 512

# Relu engine per (pass q, chunk c): 'D' = DVE tensor_scalar_max
# [128,2048] (4x), 'A' = ACT [128,2048] relu. 11 DVE quarters, 21 ACT.
# Last quarter of the kernel is D so the tail chain is short. Balance:
# DVE = 32x1.14 + 11x0.68 + evac ~= 44.5; ACT = 21x2.0 + evac ~= 43.3.
RELU_ENG = [
    "ADAADAAD",
    "AADAADAA",
    "DAADAADA",
    "ADAADAAD",
]

_CACHE = {}


def _build_program():
    nc = bacc.Bacc("TRN2", target_bir_lowering=False, debug=False,
                   num_devices=N_CORES)

    # ---- DRAM I/O ----
    blob_d = nc.dram_tensor("blob", [128, BLOB_W], bf16,
                            kind="ExternalInput").ap()
    u0_d = nc.dram_tensor("u0", [128, 512], bf16, kind="ExternalInput").ap()
    u1_d = nc.dram_tensor("u1", [128, 1536], bf16, kind="ExternalInput").ap()
    u2_d = nc.dram_tensor("u2", [128, 2048], bf16, kind="ExternalInput").ap()
    u3_d = nc.dram_tensor("u3", [128, 2048], bf16, kind="ExternalInput").ap()
    u4_d = nc.dram_tensor("u4", [128, 2048], bf16, kind="ExternalInput").ap()
    etp_d = nc.dram_tensor("etp", [128, N], bf16,
                           kind="ExternalInput").ap()
    rinv_d = nc.dram_tensor("rinv", [128, 1], f32, kind="ExternalInput").ap()
    o_d = nc.dram_tensor("o", [128, 2048], bf16, kind="ExternalOutput").ap()

    with tile.TileContext(nc) as tc, ExitStack() as ctx:
        cons = ctx.enter_context(tc.tile_pool(name="cons", bufs=1))
        zp = ctx.enter_context(tc.tile_pool(name="zp", bufs=4))
        rp = ctx.enter_context(tc.tile_pool(name="rp", bufs=6))
        accp = ctx.enter_context(tc.tile_pool(name="accp", bufs=1, space="PSUM"))

        blob = cons.tile([128, BLOB_W], bf16)
        urep = cons.tile([128, F_FULL], bf16)
        etp = cons.tile([128, N], bf16)
        r_t = cons.tile([ROWS, 1], f32)

        # ---- DMAs on three queues, ordered by need-time. u cols 0:512
        # are uploaded twice (blob for chunk 0's first sub-adds, urep
        # for the rest) so every quarter AP stays within one tile. The
        # gpsimd SWDGE queue measures ~2.4x faster than the HWDGE
        # queues (~240 vs ~100 GB/s), so it carries the bulk urep.
        nc.sync.dma_start(blob[:], blob_d[:, :])
        nc.sync.dma_start(urep[:, 0:512], u0_d[:, :])
        nc.gpsimd.dma_start(urep[:, 512:2048], u1_d[:, :])
        nc.gpsimd.dma_start(urep[:, 2048:4096], u2_d[:, :])
        nc.gpsimd.dma_start(urep[:, 4096:6144], u3_d[:, :])
        nc.gpsimd.dma_start(urep[:, 6144:8192], u4_d[:, :])
        nc.scalar.dma_start(etp[:], etp_d[:, :])
        nc.scalar.dma_start(r_t[:], rinv_d[:, :])

        ys_jp = blob[:, 0:YW]
        et_all = etp[:, 0:N]

        def usl(c0, c1, from_blob=False):
            # u columns [c0, c1): chunk 0's first sub-adds read the
            # early blob copy, everything else the full urep tile
            if from_blob and c1 <= 512:
                return blob[:, YW + c0:YW + c1]
            return urep[:, c0:c1]

        t_accs = [accp.tile([128, 512], f32, tag=f"acc{n2}", name=f"t_acc{n2}")
                  for n2 in range(4)]
        fill_b = accp.tile([128, 512], f32, tag="fill", name="fill_b")
        t_sb = cons.tile([128, 2048], bf16)

        # preload ACT's relu table during the DMA head so the first
        # real relu doesn't pay the ~1.3us ACT_TABLE_LOAD mid-kernel
        nc.scalar.activation(t_sb[0:1, 0:1], t_sb[0:1, 0:1], AF.Relu)

        def emit_add(c, z, q, parts):
            # z[:, zl] = ys_c (bcast over i) + u[qcols], in sub-steps
            ys_c = ys_jp[:, DO * c:DO * (c + 1)]
            pos = 0
            for step in parts:
                sl = (QUART * q + pos, QUART * q + pos + step)
                ys_b = ys_c.rearrange("p d -> p () d").broadcast_to(
                    (128, step // DO, DO))
                zv = z[:, pos:pos + step].rearrange(
                    "p (i d) -> p i d", i=step // DO)
                uv = usl(*sl, from_blob=(c == 0 and q == 0)).rearrange(
                    "p (i d) -> p i d", i=step // DO)
                nc.vector.tensor_tensor(zv, ys_b, uv, ALU.add)
                pos += step

        def emit_quarter(q, c, first, last):
            z = zp.tile([128, QUART], bf16, name="z")
            subs = (512, 512, 1024) if (q, c) == (0, 0) else (QUART,)
            emit_add(c, z, q, subs)
            # PE keep-warm filler: fires as soon as z (pre-relu) exists,
            # bridging the idle window while the relu runs.
            nc.tensor.matmul(fill_b[0:32, :], et_all[:, 0:32], z[:, 0:512],
                             start=True, stop=True, skip_group_check=True)
            r = rp.tile([128, QUART], bf16, name="r")
            if RELU_ENG[q][c] == "D":
                nc.vector.tensor_scalar_max(r[:], z[:], 0.0)
            else:
                nc.scalar.activation(r[:], z[:], AF.Relu)
            for n2 in range(4):
                nc.tensor.matmul(
                    t_accs[n2][32 * q:32 * (q + 1), :],
                    et_all[:, 128 * c + 32 * q:128 * c + 32 * q + 32],
                    r[:, 512 * n2:512 * (n2 + 1)],
                    start=first,
                    stop=last,
                    skip_group_check=True,
                    tile_position=(0, 32 * q),
                )
                if last and q == 3:
                    # bank n2 fully accumulated: scaled evacuation
                    # (DVE/ACT alternating); bank pairs stream out as
                    # one 2KB-row DMA on the fast gpsimd queue
                    osl = slice(512 * n2, 512 * (n2 + 1))
                    if n2 % 2 == 0:
                        nc.vector.tensor_scalar_mul(t_sb[:, osl],
                                                    t_accs[n2][:, :], r_t[:])
                    else:
                        nc.scalar.activation(t_sb[:, osl], t_accs[n2][:, :],
                                             AF.Copy, bias=0.0, scale=r_t[:])
                        psl = slice(512 * (n2 - 1), 512 * (n2 + 1))
                        nc.gpsimd.dma_start(out=o_d[:, psl],
                                            in_=t_sb[:, psl])

        for q in range(4):
            for c in range(NCHUNK):
                emit_quarter(q, c, first=(c == 0), last=(c == NCHUNK - 1))

    nc.compile()
    return nc


def _prep_inputs(x, adj, Wf, bf_, Ww, bw):
    b = ml_dtypes.bfloat16
    x64 = x.astype(np.float64)
    ys = (x64 @ Wf[:, :DI].astype(np.float64).T).astype(np.float32)   # [N, 64]
    u = (x64 @ Wf[:, DI:].astype(np.float64).T + bf_).astype(np.float32)
    asrc = (x64 @ Ww[0, :DI].astype(np.float64)).astype(np.float32)   # [N]
    g = np.exp(asrc.astype(np.float64)).astype(np.float32)            # [N]

    # ysjp[jl, 64c+d] = ys[128c+jl, d]
    ysjp = ys.reshape(NCHUNK, 128, DO).transpose(1, 0, 2).reshape(128, -1)
    # e'^T[j, i] = g[j] * (adj[i, j] > 0), chunk-packed:
    # etp[jl, 128c+il] = e'^T[128c+jl, il]
    mask_t = (adj > 0).T.astype(np.float32)          # [j, i]
    et_full = mask_t * g[:, None]                    # [j, i]
    sfull = et_full.sum(axis=0)                      # [i] row sums (denom)

    in_maps = []
    for c in range(N_CORES):
        blk = slice(ROWS * c, ROWS * (c + 1))
        et = et_full[:, blk]                          # [1024, 128]
        etp = et.reshape(NCHUNK, 128, ROWS).transpose(1, 0, 2).reshape(128, -1)
        uflat = u[blk].reshape(F_FULL).astype(b)      # [8192]
        ubc = np.ascontiguousarray(
            np.broadcast_to(uflat, (128, F_FULL)))    # host-side replicate
        blob = np.empty((128, BLOB_W), b)
        blob[:, 0:YW] = ysjp.astype(b)
        blob[:, YW:] = ubc[:, 0:512]
        m = dict(
            blob=blob,
            u0=np.ascontiguousarray(ubc[:, 0:512]),
            u1=np.ascontiguousarray(ubc[:, 512:2048]),
            u2=np.ascontiguousarray(ubc[:, 2048:4096]),
            u3=np.ascontiguousarray(ubc[:, 4096:6144]),
            u4=np.ascontiguousarray(ubc[:, 6144:8192]),
            etp=np.ascontiguousarray(etp).astype(b),
            rinv=np.ascontiguousarray(
                (1.0 / sfull[blk]).reshape(128, 1)).astype(np.float32),
        )
        in_maps.append(m)
    return in_maps


def get_program():
    if "nc" not in _CACHE:
        _CACHE["nc"] = _build_program()
    return _CACHE["nc"]


def unpack_output(res_list):
    p_idx = np.arange(128)
    col0 = (p_idx % 32) * DO
    cols = col0[:, None] + np.arange(DO)[None, :]
    out = np.empty((N, DO), np.float32)
    for c in range(N_CORES):
        t = res_list[c]["o"].astype(np.float32)      # [128, 2048]
        out[ROWS * c:ROWS * (c + 1)] = t[p_idx[:, None], cols]
    return out


def kernel(x, adj, Wf, bf, Ww, bw):
    x = np.asarray(x, dtype=np.float32)
    adj = np.asarray(adj, dtype=np.int32)
    Wf = np.asarray(Wf, dtype=np.float32)
    bf_ = np.asarray(bf, dtype=np.float32)
    Ww = np.asarray(Ww, dtype=np.float32)
    bw = np.asarray(bw, dtype=np.float32)
    assert x.shape == (N, DI) and adj.shape == (N, N)

    nc = get_program()
    in_maps = _prep_inputs(x, adj, Wf, bf_, Ww, bw)
    res = run_bass_kernel_spmd(nc, in_maps, core_ids=list(range(N_CORES)))
    return unpack_output(res.results)
